# revision 30
# baseline (speedup 1.0000x reference)
"""GCN link predictor on 8 TRN2 NeuronCores.

Strategy (1D node partition, dst-sharded SPMM, pull-mode gathers):
  - h1 = x @ W1 computed sharded (98 node-blocks = 12544 rows per core),
    written as 256B rows and AllGathered straight into the full gather
    table h1pad [100352, 128] bf16 (no separate expand pass).
  - adjacency edges are sharded by dst owner (12544 nodes/core, aligned
    with the AllGather concat); within a core, edges are grouped by
    (128-row dst block, 32768-row src chunk) so each dma_gather call uses
    int16 indices against a single chunk base. Gather calls round-robin
    over 4 SWDGE queues (3.75x DMA-gather parallelism, ~2.1ns/descriptor).
  - segment-sum = PE matmul with HOST-PRECOMPUTED one-hot tiles streamed
    from HBM (oh[e, d] = (dst_local[e]==d) * val[e], bf16 [128,128] per
    128-edge tile, stored partition-major for contiguous slab loads) —
    no on-device one-hot build, DVE/Act stay free.
  - compute-dependent stores issue from the Act engine DGE (nc.scalar)
    so the SP queue stays input-only and prefetch never stalls.
  - layer 2: h2 = z1 @ W2 locally, AllGather (pre-expanded, as above)
    into table2, same SPMM reusing the SAME one-hot stream.
  - z2 AllGathered (bf16, pre-expanded) into table3; decoder edges
    sharded by edge, grouped by (src chunk, dst chunk), two gathers per
    call batch (on rotating queues) + DVE mult/reduce per tile.
"""
import sys
import os

sys.path.insert(0, "/opt/trn_rl_repo")

import numpy as np
import ml_dtypes
from contextlib import ExitStack

from concourse import bass, bacc, tile, bass_utils
import concourse.mybir as mybir


def _install_ntff_hook():
    """Provide antenv.axon_hooks (missing in this image) so that
    run_bass_kernel_spmd(trace=True) can capture NTFF profiles via the
    axon PJRT .so — mirrors trn_agent_boot's ctypes shim."""
    if "antenv.axon_hooks" in sys.modules:
        return
    import types, ctypes, contextlib
    import antenv

    mod = types.ModuleType("antenv.axon_hooks")
    holder = {}
    mod.set_axon_ntff_profile_hook = lambda h: holder.__setitem__("h", h)
    mod.get_axon_ntff_profile_hook = lambda: holder.get("h")
    sys.modules["antenv.axon_hooks"] = mod
    antenv.axon_hooks = mod

    so_path = "/opt/axon/libaxon_pjrt.so"
    if not os.path.exists(so_path):
        return
    lib = ctypes.CDLL(so_path)
    if not hasattr(lib, "axon_start_nrt_profile"):
        return
    lib.axon_start_nrt_profile.argtypes = [ctypes.POINTER(ctypes.c_int64),
                                           ctypes.c_size_t]
    lib.axon_start_nrt_profile.restype = ctypes.c_int64
    lib.axon_stop_nrt_profile.argtypes = [ctypes.c_char_p]
    lib.axon_stop_nrt_profile.restype = ctypes.c_int64

    @contextlib.contextmanager
    def _hook(output_dir, device_ids):
        import jax
        jax.devices()
        if device_ids:
            ids = (ctypes.c_int64 * len(device_ids))(*device_ids)
            rc = lib.axon_start_nrt_profile(ids, len(device_ids))
        else:
            rc = lib.axon_start_nrt_profile(None, 0)
        if rc != 0:
            raise RuntimeError(f"axon_start_nrt_profile rc={rc}")
        try:
            yield
        finally:
            n = lib.axon_stop_nrt_profile(str(output_dir).encode())
            print(f"profile: {n} file(s) written to {output_dir}",
                  file=sys.stderr)

    mod.set_axon_ntff_profile_hook(_hook)


_install_ntff_hook()

F32 = mybir.dt.float32
BF16 = mybir.dt.bfloat16
I16 = mybir.dt.int16
BF = ml_dtypes.bfloat16

N_NODES = 100000
D_IN = 256
D_HID = 64
D_EMB = 32
ADJ_NNZ = 3200000
N_EDGES = 2000000
NCORE = 8
NODES_PAD2 = 8 * 98 * 128           # 100352 (8 equal shards of 98 blocks)
ZROWS = 98 * 128                    # 12544 per-core rows; dst shard == ZROWS
PSHARD = ZROWS                      # dst-shard size (aligns AllGather concat)
NBLK = 98                           # dst blocks per core (last has 84 rows)
CHUNK = 32768
NCHUNK = 4
SUPER = 4                           # dst blocks per superblock
NSUPER = (NBLK + SUPER - 1) // SUPER
DEC_CALL_TILES = 24                 # decoder edges per gather call = 24*128
NQUEUES = int(os.environ.get("NQUEUES") or "4")

LAST_RESULT = None                  # BassKernelResults of the last run


def _wrap16(idx):
    """idx j -> partition j%16, col j//16, replicated to 128 partitions."""
    n = len(idx)
    assert n % 16 == 0
    a = idx.reshape(n // 16, 16).T
    return np.tile(a, (8, 1)).astype(np.int16)


def _prep_spmm(adj_src, adj_dst, adj_val):
    """Shard + sort adjacency; build per-core gather idx + one-hot streams.

    Returns (T_bc [98,4] common tile counts, NT, per-core dict with idx_w
    and oh [NT*128, 128] bf16).
    """
    owner = adj_dst // PSHARD
    cores = []
    for m in range(NCORE):
        sel = owner == m
        src = adj_src[sel].astype(np.int64)
        ldst = (adj_dst[sel] - m * PSHARD).astype(np.int64)
        val = adj_val[sel]
        blk = ldst >> 7
        chk = src // CHUNK
        order = np.lexsort((chk, blk))
        src, ldst, val, blk, chk = (a[order] for a in (src, ldst, val, blk, chk))
        key = blk * NCHUNK + chk
        cnt = np.bincount(key, minlength=NBLK * NCHUNK).reshape(NBLK, NCHUNK)
        starts = np.zeros(NBLK * NCHUNK + 1, np.int64)
        np.cumsum(cnt.ravel(), out=starts[1:])
        cores.append(dict(src=src, ldst=ldst, val=val, cnt=cnt, starts=starts))

    cnt_max = np.maximum.reduce([c["cnt"] for c in cores])
    T_bc = -(-cnt_max // 128)  # ceil
    NT = int(T_bc.sum())

    for c in cores:
        idx_stream = np.zeros(NT * 128, np.int16)
        pos_l, dcol_l, val_l = [], [], []
        pos = 0
        for sb in range(NSUPER):
            blocks = range(sb * SUPER, min((sb + 1) * SUPER, NBLK))
            for ch in range(NCHUNK):
                for b in blocks:
                    t = int(T_bc[b, ch])
                    if t == 0:
                        continue
                    s = c["starts"][b * NCHUNK + ch]
                    e = c["starts"][b * NCHUNK + ch + 1]
                    n = e - s
                    idx_stream[pos:pos + n] = (c["src"][s:e] - ch * CHUNK).astype(np.int16)
                    pos_l.append(pos + np.arange(n))
                    dcol_l.append(c["ldst"][s:e] & 127)
                    val_l.append(c["val"][s:e])
                    pos += t * 128
        assert pos == NT * 128
        oh = np.zeros((NT * 128, 128), BF)
        oh[np.concatenate(pos_l), np.concatenate(dcol_l)] = np.concatenate(val_l).astype(BF)
        # transpose to [128 partitions, NT*128]: partition = edge slot within
        # tile, free dim = (tile, dstcol) -> slab loads become contiguous
        c["oh"] = np.ascontiguousarray(
            oh.reshape(NT, 128, 128).transpose(1, 0, 2).reshape(128, NT * 128))
        # wrap idx per call (call = (sb, ch) contiguous span)
        cols = []
        p = 0
        for sb in range(NSUPER):
            blocks = range(sb * SUPER, min((sb + 1) * SUPER, NBLK))
            for ch in range(NCHUNK):
                t = int(T_bc[list(blocks), ch].sum())
                if t == 0:
                    continue
                cols.append(_wrap16(idx_stream[p:p + t * 128]))
                p += t * 128
        c["idx_w"] = np.concatenate(cols, axis=1)
        for k in ("src", "ldst", "val", "cnt", "starts"):
            del c[k]
    return T_bc, NT, cores


def _prep_decoder(edge_index):
    """Shard decoder edges by edge id; group by (src chunk, dst chunk)."""
    per = N_EDGES // NCORE
    cores = []
    for m in range(NCORE):
        src = edge_index[0, m * per:(m + 1) * per].astype(np.int64)
        dst = edge_index[1, m * per:(m + 1) * per].astype(np.int64)
        g = (src // CHUNK) * NCHUNK + dst // CHUNK
        order = np.argsort(g, kind="stable")
        src, dst, g = src[order], dst[order], g[order]
        eid = m * per + order  # global edge ids in stream order
        cnt = np.bincount(g, minlength=16)
        starts = np.zeros(17, np.int64)
        np.cumsum(cnt, out=starts[1:])
        cores.append(dict(src=src, dst=dst, eid=eid, cnt=cnt, starts=starts))

    cnt_max = np.maximum.reduce([c["cnt"] for c in cores])
    T_g = -(-cnt_max // 128)
    DEC_NT = int(T_g.sum())
    # call list: (group, col_base(tiles), tiles)
    calls = []
    base = 0
    for g in range(16):
        t = int(T_g[g])
        off = 0
        while off < t:
            tt = min(DEC_CALL_TILES, t - off)
            calls.append((g, base + off, tt))
            off += tt
        base += t

    for c in cores:
        src_s = np.zeros(DEC_NT * 128, np.int16)
        dst_s = np.zeros(DEC_NT * 128, np.int16)
        emap = np.full(DEC_NT * 128, -1, np.int64)
        pos = 0
        for g in range(16):
            s, e = c["starts"][g], c["starts"][g + 1]
            n = e - s
            gs, gd = g // NCHUNK, g % NCHUNK
            src_s[pos:pos + n] = (c["src"][s:e] - gs * CHUNK).astype(np.int16)
            dst_s[pos:pos + n] = (c["dst"][s:e] - gd * CHUNK).astype(np.int16)
            emap[pos:pos + n] = c["eid"][s:e]
            pos += int(T_g[g]) * 128
        assert pos == DEC_NT * 128
        # wrap per call
        sc, dc = [], []
        for (g, cb, tt) in calls:
            span = slice(cb * 128, (cb + tt) * 128)
            sc.append(_wrap16(src_s[span]))
            dc.append(_wrap16(dst_s[span]))
        c["src_w"] = np.concatenate(sc, axis=1)
        c["dst_w"] = np.concatenate(dc, axis=1)
        # emap as [128, DEC_NT]: value at [p, t] is edge at pos t*128+p
        c["emap"] = emap.reshape(DEC_NT, 128).T.copy()
        for k in ("src", "dst", "eid", "cnt", "starts"):
            del c[k]
    return T_g, DEC_NT, calls, cores


def _build(T_bc, NT, DEC_NT, dec_calls, idx_cols, dec_cols):
    nc = bacc.Bacc("TRN2", target_bir_lowering=False, debug=False,
                   enable_asserts=True, num_devices=NCORE,
                   num_swdge_queues=NQUEUES)

    xTs_d = nc.dram_tensor("xTs", [D_IN, ZROWS], BF16, kind="ExternalInput")
    w1_d = nc.dram_tensor("W1", [D_IN, D_HID], BF16, kind="ExternalInput")
    w2_d = nc.dram_tensor("W2", [D_HID, D_EMB], BF16, kind="ExternalInput")
    b1t_d = nc.dram_tensor("b1t", [128, D_HID], BF16, kind="ExternalInput")
    b2t_d = nc.dram_tensor("b2t", [128, D_EMB], BF16, kind="ExternalInput")
    ident_d = nc.dram_tensor("ident", [128, 128], BF16, kind="ExternalInput")
    idx_d = nc.dram_tensor("idx", [128, idx_cols], I16, kind="ExternalInput")
    ohm_d = nc.dram_tensor("ohm", [128, NT * 128], BF16, kind="ExternalInput")
    dsrc_d = nc.dram_tensor("dsrc", [128, dec_cols], I16, kind="ExternalInput")
    ddst_d = nc.dram_tensor("ddst", [128, dec_cols], I16, kind="ExternalInput")
    scores_d = nc.dram_tensor("scores", [128, DEC_NT], F32, kind="ExternalOutput")

    # internal DRAM — per-core shard written 256B-row pre-expanded, then
    # AllGathered straight into the full-size gather tables.
    h1loc_d = nc.dram_tensor("h1loc", [ZROWS, 128], BF16, kind="Internal")
    h1pad_d = nc.dram_tensor("h1pad", [NODES_PAD2, 128], BF16,
                             kind="Internal", addr_space="Shared")
    z1_d = nc.dram_tensor("z1", [ZROWS, 128], BF16, kind="Internal")
    h2loc_d = nc.dram_tensor("h2loc", [ZROWS, 128], BF16, kind="Internal")
    table2_d = nc.dram_tensor("table2", [NODES_PAD2, 128], BF16,
                              kind="Internal", addr_space="Shared")
    z2loc_d = nc.dram_tensor("z2loc", [ZROWS, 128], BF16, kind="Internal")
    table3_d = nc.dram_tensor("table3", [NODES_PAD2, 128], BF16,
                              kind="Internal", addr_space="Shared")

    rg = [list(range(NCORE))]
    qrr = [0]

    def next_q():
        q = qrr[0] % NQUEUES
        qrr[0] += 1
        return q

    # per-call tile counts for spmm gathers
    def spmm_calls():
        out = []
        for sb in range(NSUPER):
            blocks = list(range(sb * SUPER, min((sb + 1) * SUPER, NBLK)))
            for ch in range(NCHUNK):
                t = int(T_bc[blocks, ch].sum())
                if t:
                    out.append((sb, ch, blocks, t))
        return out

    CALLS = spmm_calls()
    call_tile_base = {}
    tb = 0
    for (sb, ch, blocks, t) in CALLS:
        call_tile_base[(sb, ch)] = tb
        tb += t
    assert tb == NT

    def spmm_phase(tc, pool, table_ap, out_w, bias_tile, relu, out_dtype,
                   z_out_d, tag):
        nc_ = tc.nc
        with ExitStack() as ctx:
            lp = ctx.enter_context(tc.tile_pool(name=f"sp_{tag}", bufs=3))
            op_ = ctx.enter_context(tc.tile_pool(name=f"oh_{tag}", bufs=3))
            gp = ctx.enter_context(tc.tile_pool(name=f"g_{tag}", bufs=6))
            pp = ctx.enter_context(
                tc.tile_pool(name=f"ps_{tag}", bufs=6, space="PSUM"))
            ident_sb = pool.tile([128, 128], BF16, tag="ident")
            bt_sb = pool.tile([128, out_w], BF16, tag=f"bt_{tag}")
            nc_.sync.dma_start(ident_sb[:], ident_d[:])
            nc_.sync.dma_start(bt_sb[:], bias_tile[:])

            icol = 0
            for sb in range(NSUPER):
                blocks = list(range(sb * SUPER, min((sb + 1) * SUPER, NBLK)))
                sb_tiles = int(T_bc[blocks, :].sum())
                sb_tile0 = call_tile_base[(sb, [ch for ch in range(NCHUNK)
                                                if (sb, ch) in call_tile_base][0])]
                # streamed one-hot slab for this superblock (contiguous read;
                # alternate issuing queue so loads pipeline two-deep)
                oh_sb = op_.tile([128, sb_tiles, 128], BF16, tag="ohs")
                eng = nc_.sync if sb % 2 == 0 else nc_.scalar
                eng.dma_start(
                    oh_sb[:],
                    ohm_d[:, sb_tile0 * 128:(sb_tile0 + sb_tiles) * 128]
                    .rearrange("p (j f) -> p j f", f=128))
                idx_sb = lp.tile([128, sb_tiles * 8], I16, tag="idx")
                nc_.sync.dma_start(idx_sb[:], idx_d[:, icol:icol + sb_tiles * 8])

                gath = {}
                ic_local = 0
                for ch in range(NCHUNK):
                    if (sb, ch) not in call_tile_base:
                        continue
                    t = int(T_bc[blocks, ch].sum())
                    rows = min(CHUNK, NODES_PAD2 - ch * CHUNK)
                    g = gp.tile([128, t, 128], BF16, tag="gath")
                    nc_.gpsimd.dma_gather(
                        out_ap=g[:],
                        in_ap=table_ap[ch * CHUNK:ch * CHUNK + rows, :],
                        idxs_ap=idx_sb[:, ic_local:ic_local + t * 8],
                        num_idxs=t * 128,
                        num_idxs_reg=t * 128,
                        elem_size=128,
                        single_packet=False,
                        queue_num=next_q(),
                    )
                    gath[ch] = g
                    ic_local += t * 8
                icol += sb_tiles * 8

                zwide = lp.tile([128, len(blocks), 128], out_dtype, tag="zw")
                for bi, b in enumerate(blocks):
                    ps = pp.tile([128, out_w], F32, tag="ps")
                    first = True
                    for ch in range(NCHUNK):
                        if (sb, ch) not in call_tile_base:
                            continue
                        off = int(T_bc[blocks[:bi], ch].sum()) if bi else 0
                        gtile0 = call_tile_base[(sb, ch)] + off
                        for ti in range(int(T_bc[b, ch])):
                            mcol = gtile0 + ti - sb_tile0
                            nc_.tensor.matmul(
                                ps[:], oh_sb[:, mcol, :],
                                gath[ch][:, off + ti, 0:out_w],
                                start=first, stop=False)
                            first = False
                    nc_.tensor.matmul(ps[:], ident_sb[:], bt_sb[:],
                                      start=first, stop=True)
                    func = (mybir.ActivationFunctionType.Relu if relu
                            else mybir.ActivationFunctionType.Copy)
                    nc_.scalar.activation(zwide[:, bi, 0:out_w], ps[:], func)
                # store from the Act engine's DGE so the Sync queue stays
                # input-only and the next superblock's oh/idx prefetch is
                # not blocked behind this compute-dependent write
                nc_.scalar.dma_start(
                    z_out_d[sb * SUPER * 128:
                            (sb * SUPER + len(blocks)) * 128, :]
                    .rearrange("(j p) f -> p j f", p=128),
                    zwide[:],
                )

    with tile.TileContext(nc) as tc:
        with ExitStack() as octx:
            pool = octx.enter_context(tc.tile_pool(name="const", bufs=1))

            # ---- Phase A: h1 shard = x[shard] @ W1; AllGather; expand ----
            with ExitStack() as ctx:
                ap = ctx.enter_context(tc.tile_pool(name="pA", bufs=3))
                app = ctx.enter_context(
                    tc.tile_pool(name="pAp", bufs=8, space="PSUM"))
                w1_sb = pool.tile([128, 2, D_HID], BF16, tag="w1")
                nc.sync.dma_start(
                    w1_sb[:], w1_d[:].rearrange("(k p) f -> p k f", p=128))
                PB = 7  # node blocks per panel; 98 = 14*7
                for p0 in range(0, NBLK, PB):
                    nb = min(PB, NBLK - p0)
                    n0 = p0 * 128
                    xt0 = ap.tile([128, nb * 128], BF16, tag="xt0")
                    xt1 = ap.tile([128, nb * 128], BF16, tag="xt1")
                    nc.sync.dma_start(xt0[:], xTs_d[0:128, n0:n0 + nb * 128])
                    nc.sync.dma_start(xt1[:], xTs_d[128:256, n0:n0 + nb * 128])
                    hw = ap.tile([128, nb, 128], BF16, tag="hw")
                    for j in range(nb):
                        ps = app.tile([128, D_HID], F32, tag="psA")
                        nc.tensor.matmul(ps[:], xt0[:, j * 128:(j + 1) * 128],
                                         w1_sb[:, 0, :], start=True, stop=False)
                        nc.tensor.matmul(ps[:], xt1[:, j * 128:(j + 1) * 128],
                                         w1_sb[:, 1, :], start=False, stop=True)
                        nc.scalar.activation(hw[:, j, 0:D_HID], ps[:],
                                             mybir.ActivationFunctionType.Copy)
                    nc.scalar.dma_start(
                        h1loc_d[n0:n0 + nb * 128, :]
                        .rearrange("(j p) f -> p j f", p=128),
                        hw[:],
                    )
                nc.gpsimd.collective_compute(
                    "AllGather", mybir.AluOpType.bypass, replica_groups=rg,
                    ins=[h1loc_d[:]], outs=[h1pad_d.ap()])

            # ---- Phase B: SPMM1 -> z1 (relu) ----
            spmm_phase(tc, pool, h1pad_d[:], D_HID, b1t_d, True, BF16,
                       z1_d, "b")

            # ---- Phase C: h2 = z1 @ W2; AllGather; expand to table2 ----
            with ExitStack() as ctx:
                cp = ctx.enter_context(tc.tile_pool(name="pC", bufs=3))
                cpp = ctx.enter_context(
                    tc.tile_pool(name="pCp", bufs=4, space="PSUM"))
                z1T = cp.tile([128, ZROWS], BF16, tag="z1T")
                nc.sync.dma_start(z1T[:], z1_d[:], transpose=True)
                w2_sb = pool.tile([D_HID, D_EMB], BF16, tag="w2")
                nc.sync.dma_start(w2_sb[:], w2_d[:])
                PB = 8
                for p0 in range(0, NBLK, PB):
                    nb = min(PB, NBLK - p0)
                    hw = cp.tile([128, nb, 128], BF16, tag="h2w")
                    for j in range(nb):
                        b = p0 + j
                        ps = cpp.tile([128, D_EMB], F32, tag="psC")
                        nc.tensor.matmul(ps[:], z1T[0:D_HID, b * 128:(b + 1) * 128],
                                         w2_sb[:], start=True, stop=True)
                        nc.scalar.activation(hw[:, j, 0:D_EMB], ps[:],
                                             mybir.ActivationFunctionType.Copy)
                    nc.scalar.dma_start(
                        h2loc_d[p0 * 128:(p0 + nb) * 128, :]
                        .rearrange("(j p) f -> p j f", p=128),
                        hw[:],
                    )
                nc.gpsimd.collective_compute(
                    "AllGather", mybir.AluOpType.bypass, replica_groups=rg,
                    ins=[h2loc_d[:]], outs=[table2_d.ap()])

            # ---- Phase D: SPMM2 -> z2 (no relu, bf16 out) ----
            spmm_phase(tc, pool, table2_d[:], D_EMB, b2t_d, False, BF16,
                       z2loc_d, "d")

            # AllGather z2 straight into the decoder gather table
            nc.gpsimd.collective_compute(
                "AllGather", mybir.AluOpType.bypass, replica_groups=rg,
                ins=[z2loc_d[:]], outs=[table3_d.ap()])

            # ---- Phase E: decoder ----
            with ExitStack() as ctx:
                ep = ctx.enter_context(tc.tile_pool(name="pE", bufs=4))
                icol = 0
                for (g, cb, tt) in dec_calls:
                    gs, gd = g // NCHUNK, g % NCHUNK
                    si = ep.tile([128, tt * 8], I16, tag="si")
                    di = ep.tile([128, tt * 8], I16, tag="di")
                    nc.sync.dma_start(si[:], dsrc_d[:, icol:icol + tt * 8])
                    nc.sync.dma_start(di[:], ddst_d[:, icol:icol + tt * 8])
                    icol += tt * 8
                    A = ep.tile([128, tt, 128], BF16, tag="A")
                    B = ep.tile([128, tt, 128], BF16, tag="B")
                    rows_s = min(CHUNK, NODES_PAD2 - gs * CHUNK)
                    rows_d = min(CHUNK, NODES_PAD2 - gd * CHUNK)
                    nc.gpsimd.dma_gather(
                        out_ap=A[:], in_ap=table3_d[gs * CHUNK:gs * CHUNK + rows_s, :],
                        idxs_ap=si[:], num_idxs=tt * 128, num_idxs_reg=tt * 128,
                        elem_size=128, single_packet=False, queue_num=next_q())
                    nc.gpsimd.dma_gather(
                        out_ap=B[:], in_ap=table3_d[gd * CHUNK:gd * CHUNK + rows_d, :],
                        idxs_ap=di[:], num_idxs=tt * 128, num_idxs_reg=tt * 128,
                        elem_size=128, single_packet=False, queue_num=next_q())
                    prod = ep.tile([128, tt, D_EMB], F32, tag="prod")
                    nc.vector.tensor_tensor(
                        prod[:], A[:, :, 0:D_EMB], B[:, :, 0:D_EMB],
                        mybir.AluOpType.mult)
                    red = ep.tile([128, tt], F32, tag="red")
                    nc.vector.tensor_reduce(
                        red[:], prod[:], mybir.AxisListType.X,
                        mybir.AluOpType.add)
                    nc.scalar.dma_start(scores_d[:, cb:cb + tt], red[:])

    nc.compile()
    return nc


def kernel(x, adj_src, adj_dst, adj_val, edge_index, W1, b1, W2, b2):
    global LAST_RESULT
    x = np.asarray(x, np.float32)
    adj_src = np.asarray(adj_src, np.int32)
    adj_dst = np.asarray(adj_dst, np.int32)
    adj_val = np.asarray(adj_val, np.float32)
    edge_index = np.asarray(edge_index, np.int32)
    W1 = np.asarray(W1, np.float32)
    b1 = np.asarray(b1, np.float32)
    W2 = np.asarray(W2, np.float32)
    b2 = np.asarray(b2, np.float32)

    T_bc, NT, spmm_cores = _prep_spmm(adj_src, adj_dst, adj_val)
    T_g, DEC_NT, dec_calls, dec_cores = _prep_decoder(edge_index)
    idx_cols = spmm_cores[0]["idx_w"].shape[1]
    dec_cols = dec_cores[0]["src_w"].shape[1]

    nc = _build(T_bc, NT, DEC_NT, dec_calls, idx_cols, dec_cols)

    xT = np.zeros((D_IN, NODES_PAD2), BF)
    xT[:, :N_NODES] = x.T.astype(BF)
    ident = np.eye(128, dtype=BF)
    b1t = np.tile(b1.astype(BF)[None, :], (128, 1))
    b2t = np.tile(b2.astype(BF)[None, :], (128, 1))
    common = {
        "W1": W1.astype(BF), "W2": W2.astype(BF),
        "b1t": b1t, "b2t": b2t, "ident": ident,
    }
    in_maps = []
    for m in range(NCORE):
        in_maps.append({
            **common,
            "xTs": np.ascontiguousarray(xT[:, m * ZROWS:(m + 1) * ZROWS]),
            "idx": spmm_cores[m]["idx_w"],
            "ohm": spmm_cores[m]["oh"],
            "dsrc": dec_cores[m]["src_w"],
            "ddst": dec_cores[m]["dst_w"],
        })

    res = bass_utils.run_bass_kernel_spmd(
        nc, in_maps, core_ids=list(range(NCORE)),
        trace=bool(os.environ.get("BASS_TRACE")))
    LAST_RESULT = res

    scores = np.zeros(N_EDGES, np.float32)
    for m in range(NCORE):
        out = res.results[m]["scores"]          # [128, DEC_NT]
        emap = dec_cores[m]["emap"]             # [128, DEC_NT]
        valid = emap >= 0
        scores[emap[valid]] = out[valid]
    return scores
